# revision 32
# baseline (speedup 1.0000x reference)
"""BallQuery Trainium2 kernel.

Problem: xyz (8, 8192, 3) f32, new_xyz (8, 2048, 3) f32 -> out (8, 2048, 32) int32.
For each query row (b, m): the first 32 point indices j (ascending) with
|q - p_j|^2 < 0.1^2, padded with the first valid index; all-sentinel (8193)
when no point is in radius.

Sharding: data-parallel over batch — core i handles batch i (8 cores).

Exactness: the reference (jax CPU) computes f32 d_k = q_k - p_k, f32 squares,
and the f32 sum ((dx^2+dy^2)+dz^2) compared < r^2.  This kernel replicates
that exact rounding:
  - ACT engine: sq_k = Square(1.0*p_k + (-q_k))   (exact f32 affine + square)
  - DVE: a1 = sqx+sqy ; a2 = sqz+a1 (f32 add is commutative-exact) ;
         mask = a2 < r2 (exact compare)
Selection: running clamped count via DVE tensor_tensor_scan
(state = min(state + mask, 32), initial -1) written REVERSED as int16 ->
per-element scatter slot; GPSIMD local_scatter writes (j+1-32768) to slot
rank-1, iterating descending j so the smallest j wins each slot; min-merge
across chunks; small finalize applies the reference's padding semantics.
"""

import numpy as np

import concourse.bacc as bacc
import concourse.bass as bass
import concourse.mybir as mybir
from concourse import bass_utils
from concourse.tile import TileContext

B, N, M, NS = 8, 8192, 2048, 32
RADIUS2 = np.float32(0.1) * np.float32(0.1)
SENT = N + 1  # 8193, reference sentinel
QTR = N // 4   # 2048: n processed in four quarters (SBUF budget)
CHUNK = 1024   # local_scatter chunk
NSLOT = 34     # scatter dst slots: ranks 0..31 + trash 32 (+pad to even)
NT = M // 128  # 16 m-tiles
OFF = 32768    # int16 offset so scattered values are negative (0 = empty)

_PLAN = {}


def _build():
    if "nc" in _PLAN:
        return _PLAN["nc"]
    f32 = mybir.dt.float32
    bf16 = mybir.dt.bfloat16
    i16 = mybir.dt.int16
    i32 = mybir.dt.int32
    Alu = mybir.AluOpType
    Act = mybir.ActivationFunctionType

    nc = bacc.Bacc("TRN2", target_bir_lowering=False)
    xyz_t = nc.dram_tensor("xyz_b", [N, 3], f32, kind="ExternalInput")
    new_t = nc.dram_tensor("new_b", [M, 3], f32, kind="ExternalInput")
    out_t = nc.dram_tensor("out_b", [M, NS], i32, kind="ExternalOutput")
    pk_dram = nc.dram_tensor("pk_scratch", [3, N], f32)

    # Scatter data constants: value at reversed position p (quarter h) is
    # j + 1 - OFF with j = h*QTR + (QTR-1) - p.
    descs = []
    for h in range(4):
        row = (h * QTR + QTR - np.arange(QTR, dtype=np.int64) - OFF).astype(
            np.int16
        )
        descs.append(np.ascontiguousarray(np.broadcast_to(row, (128, QTR))))
    desc_d = [nc.inline_tensor(d, name=f"desc{h}") for h, d in enumerate(descs)]

    with TileContext(nc) as tc:
        with (
            tc.tile_pool(name="const", bufs=1) as cpool,
            tc.tile_pool(name="rep", bufs=2) as rpool,
            tc.tile_pool(name="sq", bufs=2) as sqpool,
            tc.tile_pool(name="mask", bufs=2) as mpool,
            tc.tile_pool(name="idx", bufs=2) as ipool,
            tc.tile_pool(name="fin", bufs=2) as fpool,
        ):
            # --- one-time setup ---
            with nc.allow_non_contiguous_dma(
                reason="one-time 98KB coord-split gather of xyz to DRAM scratch"
            ):
                nc.sync.dma_start(pk_dram[:], xyz_t[:].rearrange("n c -> c n"))


            q_tile = cpool.tile([128, NT * 3], f32)
            nc.sync.dma_start(
                q_tile[:, :].rearrange("p (t c) -> p t c", c=3),
                new_t[:].rearrange("(t p) c -> p t c", p=128),
            )
            negq = cpool.tile([128, NT * 3], f32)
            nc.vector.tensor_scalar(negq, q_tile, -1.0, None, Alu.mult)

            desc_s = []
            for h in range(4):
                d = cpool.tile([128, QTR], i16, tag=f"desc{h}")
                nc.sync.dma_start(d[:, :], desc_d[h][:])
                desc_s.append(d)

            c32 = cpool.tile([128, 1], bf16)
            nc.vector.memset(c32, 32.0)
            negr2 = cpool.tile([128, 1], f32)
            nc.vector.memset(negr2, -float(RADIUS2))

            # per-(tile) scatter outputs: 8 chunks x NSLOT, persistent
            dst_all = cpool.tile([128, NT * 8 * NSLOT], i16)
            carry = cpool.tile([128, NT], i16)

            # --- main pipeline ---
            for h in range(4):
                rep = []
                for k in range(3):
                    r = rpool.tile([128, QTR], f32, tag=f"rep{k}")
                    src_ap = pk_dram[k, h * QTR : (h + 1) * QTR]
                    nc.sync.dma_start(r[:, :], src_ap.partition_broadcast(128))
                    rep.append(r)

                for t in range(NT):
                    mask_h = mpool.tile([128, QTR], bf16)
                    sq = []
                    for k in range(3):
                        s = sqpool.tile([128, QTR], f32, tag=f"sq{k}")
                        nc.scalar.activation(
                            s[:, :],
                            rep[k][:, :],
                            Act.Square,
                            bias=negq[:, t * 3 + k : t * 3 + k + 1],
                            scale=1.0,
                        )
                        sq.append(s)
                    # a1 = sqx + sqy (in sq[0]); a2 = sqz + a1 (in sq[2])
                    nc.vector.tensor_add(sq[0], sq[0], sq[1])
                    nc.vector.tensor_add(sq[2], sq[2], sq[0])
                    # mask on GPSIMD (InstTensorScalarPtr is builtin ucode --
                    # no library conflict with local_scatter)
                    nc.gpsimd.tensor_scalar(
                        mask_h[:, :], sq[2], float(RADIUS2), None, Alu.is_lt
                    )

                    idxrev = ipool.tile([128, QTR], i16)
                    initial = -1.0 if h == 0 else carry[:, t : t + 1]
                    nc.vector.tensor_tensor_scan(
                        idxrev[:, ::-1],
                        mask_h[:, :],
                        c32.to_broadcast([128, QTR]),
                        initial,
                        Alu.add,
                        Alu.min,
                    )
                    if h < 3:
                        nc.gpsimd.tensor_scalar(
                            carry[:, t : t + 1], idxrev[:, 0:1], 0.0, None, Alu.add
                        )

                    for c in range(QTR // CHUNK):
                        sl = slice(c * CHUNK, (c + 1) * CHUNK)
                        di = (t * 8 + h * 2 + c) * NSLOT
                        nc.gpsimd.local_scatter(
                            dst_all[:, di : di + NSLOT],
                            desc_s[h][:, sl],
                            idxrev[:, sl],
                            channels=128,
                            num_elems=NSLOT,
                            num_idxs=CHUNK,
                        )

            # --- batched merge + finalize (strided APs over all 16 tiles) ---
            # dst_all viewed as [128, NT, 8, NSLOT]; min-tree over the 8 chunks
            d4 = dst_all[:, :].rearrange("p (t c s) -> p t c s", c=8, s=NSLOT)
            m4 = fpool.tile([128, NT * 4 * NSLOT], i16, tag="m4")
            m4v = m4[:, :].rearrange("p (t c s) -> p t c s", c=4, s=NSLOT)
            nc.vector.tensor_tensor(
                out=m4v, in0=d4[:, :, 0:4, :], in1=d4[:, :, 4:8, :], op=Alu.min
            )
            m2 = fpool.tile([128, NT * 2 * NSLOT], i16, tag="m2")
            m2v = m2[:, :].rearrange("p (t c s) -> p t c s", c=2, s=NSLOT)
            nc.vector.tensor_tensor(
                out=m2v, in0=m4v[:, :, 0:2, :], in1=m4v[:, :, 2:4, :], op=Alu.min
            )
            mg = fpool.tile([128, NT * NSLOT], i16, tag="mg")
            mgv = mg[:, :].rearrange("p (t s) -> p t s", s=NSLOT)
            nc.vector.tensor_tensor(
                out=mgv, in0=m2v[:, :, 0, :], in1=m2v[:, :, 1, :], op=Alu.min
            )

            # v = merged[:, :, :32] + (OFF-1): j for valid slots, 32767 empty
            v = fpool.tile([128, NT * NS], f32, tag="v")
            vv = v[:, :].rearrange("p (t s) -> p t s", s=NS)
            nc.gpsimd.tensor_scalar(
                vv, mgv[:, :, :NS], float(OFF - 1), None, Alu.add
            )
            e = fpool.tile([128, NT * NS], f32, tag="e")
            ev = e[:, :].rearrange("p (t s) -> p t s", s=NS)
            nc.gpsimd.tensor_scalar(ev, vv, float(OFF - 1), None, Alu.is_equal)
            a = fpool.tile([128, NT], f32, tag="a")
            nc.gpsimd.tensor_scalar(
                a, vv[:, :, 0], float(OFF - 1), None, Alu.is_equal
            )
            fs = fpool.tile([128, NT], f32, tag="fs")
            nc.vector.scalar_tensor_tensor(
                out=fs,
                in0=a,
                scalar=float(SENT - (OFF - 1)),
                in1=vv[:, :, 0],
                op0=Alu.mult,
                op1=Alu.add,
            )
            # u1 = v - fs (fs broadcast along slots); u2 = e*u1; out = v - u2
            u1 = fpool.tile([128, NT * NS], f32, tag="u1")
            u1v = u1[:, :].rearrange("p (t s) -> p t s", s=NS)
            nc.vector.tensor_tensor(
                out=u1v,
                in0=vv,
                in1=fs[:, :].to_broadcast([128, NT, NS]),
                op=Alu.subtract,
            )
            u2 = fpool.tile([128, NT * NS], f32, tag="u2")
            u2v = u2[:, :].rearrange("p (t s) -> p t s", s=NS)
            nc.vector.tensor_tensor(out=u2v, in0=ev, in1=u1v, op=Alu.mult)
            o32 = fpool.tile([128, NT * NS], i32, tag="o32")
            o32v = o32[:, :].rearrange("p (t s) -> p t s", s=NS)
            nc.vector.tensor_tensor(out=o32v, in0=vv, in1=u2v, op=Alu.subtract)

            nc.sync.dma_start(
                out_t[:].rearrange("(t p) s -> p t s", p=128), o32v
            )

    nc.compile()
    _PLAN["nc"] = nc
    return nc


def kernel(xyz: np.ndarray, new_xyz: np.ndarray) -> np.ndarray:
    xyz = np.ascontiguousarray(np.asarray(xyz, dtype=np.float32))
    new_xyz = np.ascontiguousarray(np.asarray(new_xyz, dtype=np.float32))
    nc = _build()
    in_maps = [
        {"xyz_b": xyz[b], "new_b": new_xyz[b]} for b in range(B)
    ]
    res = bass_utils.run_bass_kernel_spmd(nc, in_maps, core_ids=list(range(B)))
    return np.stack([res.results[b]["out_b"] for b in range(B)], axis=0).astype(
        np.int32
    )


if __name__ == "__main__":
    rng = np.random.default_rng(0)
    x = rng.random((B, N, 3), dtype=np.float32)
    q = rng.random((B, M, 3), dtype=np.float32)
    out = kernel(x, q)
    print(out.shape, out.dtype)
